# revision 2
# baseline (speedup 1.0000x reference)
"""Trainium2 Bass kernel: causal GQA attention (fp16 pipeline).

Problem: B=2, Sq=Sk=2048, H=32, Hkv=8, D=128, fp32 in/out, causal +
key-padding mask.

Sharding (8 cores): head-parallel. Core c takes q-heads [4c, 4c+4) for both
batches; those 4 heads share exactly one kv head (c) per batch, so each core
runs 8 independent (batch, head) pairs — K/V loaded once per batch, no comms.

All device data is fp16 (host converts): matmuls run at 1 PE cycle/row with
no minimum free-dim width (unlike fp32r's 256), DVE elementwise ops get the
2x/4x 2-byte perf modes, and DMA bytes halve. Measured numerics: ~4e-4 rel
err vs the fp32 reference (tolerance 2e-2).

Per (batch, head) pair the scores are built TRANSPOSED (keys on partitions,
queries on free) in 512-query groups, with key chunks of 128 processed in
3-chunk ST tiles (PSUM [128, 3, 512] = 3 banks):

  PE : S^T[j] = K_j @ Q_g^T   exact causal trimming (live cols only)
       diag: S^T[j] += I.T @ tri  (128-wide bias matmul, fp16)
       O^T += V_j^T @ P^T[j]      (accumulate [d=128, 512] over the group)
  ACT: P^T = exp(scale*S^T) per 3-chunk tile, ONE fused instruction
       (PSUM->SBUF, fp16 out) — ACT is the bottleneck engine; everything
       else is kept off it.
  DVE: acc += P^T[j] chunk adds (fp16, 2-byte perf modes), then the group
       epilogue: rcp = approx-recip(sums), out = O^T * rcp (PSUM read).
  Pool(gpsimd): sums = partition_all_reduce(acc) — the cross-partition
       softmax denominator — plus the output DMA ring and qt input ring.

The epilogue of group g is emitted after group g+1's adds (software
pipelining) so the in-order DVE/Pool queues never stall; PE is pipelined as
QK(t0) QK(t1) PV(t0) QK(t2) PV(t1) ... so it always has a queued matmul
while ACT exps the previous tile.

PSUM: 2 rotating 3-bank ST tiles + 2 rotating O^T banks = 8 banks.

The key-padding mask folds into the exp bias per key chunk (bias operand
indexes partitions = keys). The all-ones-mask fast path (the spec's fill)
uses fused tile exps; a non-trivial mask falls back to per-chunk biases.
"""

import math
import sys

import numpy as np

for _p in ("/opt/trn_rl_repo",):
    if _p not in sys.path:
        sys.path.append(_p)

import concourse.bass as bass
import concourse.tile as tile
from concourse import bacc, bass_isa, mybir
from concourse.bass import ts
from concourse.bass_utils import run_bass_kernel_spmd

B = 2
S = 2048
H = 32
HKV = 8
D = 128
N_CORES = 8
HPC = H // N_CORES  # q heads per core = 4
PAIRS = B * HPC  # 8 (batch, head) pairs per core
NG = S // 512  # 4 q-groups of 512 per pair
NCHUNK = S // 128  # 16 key chunks of 128
SCALE = 1.0 / math.sqrt(D)
NEG = -10000.0
TCH = 3  # key chunks per ST tile

F32 = mybir.dt.float32
F16 = mybir.dt.float16
EXP = mybir.ActivationFunctionType.Exp


def build_module(uniform_mask: bool = True):
    nc = bacc.Bacc("TRN2", target_bir_lowering=False, debug=False, num_devices=1)

    qt = nc.dram_tensor("qt", [PAIRS, D, S], F16, kind="ExternalInput").ap()
    kt = nc.dram_tensor("kt", [B, D, S], F16, kind="ExternalInput").ap()
    v = nc.dram_tensor("v", [B, S, D], F16, kind="ExternalInput").ap()
    tri = nc.dram_tensor("tri", [D, 256], F16, kind="ExternalInput").ap()
    pb = nc.dram_tensor("pb", [B, S], F32, kind="ExternalInput").ap()
    ot = nc.dram_tensor("ot", [PAIRS, NG, D, 512], F16, kind="ExternalOutput").ap()

    with tile.TileContext(nc) as tc:
        with (
            tc.tile_pool(name="consts", bufs=1) as consts,
            tc.tile_pool(name="kv", bufs=2) as kv_pool,
            tc.tile_pool(name="q", bufs=2) as q_pool,
            tc.tile_pool(name="pt", bufs=4) as pt_pool,
            tc.tile_pool(name="acc", bufs=2) as acc_pool,
            tc.tile_pool(name="rb", bufs=2) as rb_pool,
            tc.tile_pool(name="rcp", bufs=2) as rcp_pool,
            tc.tile_pool(name="osb", bufs=3) as osb_pool,
            tc.tile_pool(name="st_ps", bufs=2, space="PSUM") as st_pool,
            tc.tile_pool(name="ot_ps", bufs=2, space="PSUM") as ot_pool,
        ):
            trid_sb = consts.tile([D, 256], F16)
            nc.sync.dma_start(trid_sb[:], tri[:])
            tri_sb = trid_sb[:, :128]  # [k, q]: 0 if k<=q else NEG
            ident_sb = trid_sb[:, 128:]
            # warm the ACT exp table during the initial DMAs
            warm_in = consts.tile([1, 2], F32)
            nc.vector.memset(warm_in[:], 1.0)
            warm = consts.tile([1, 2], F32)
            nc.scalar.activation(warm[:], warm_in[:], EXP, scale=1.0)

            def _load_kv(b):
                kt_sb = kv_pool.tile([D, S], F16, tag="kt")
                v_r = v[b].rearrange("(j k) d -> k j d", k=128)
                v_sb = kv_pool.tile([D, NCHUNK, D], F16, tag="v")
                nc.sync.dma_start(kt_sb[:, ts(0, 512)], kt[b][:, ts(0, 512)])
                nc.sync.dma_start(v_sb[:, ts(0, 4), :], v_r[:, ts(0, 4), :])
                for q4 in range(1, 4):
                    nc.sync.dma_start(
                        kt_sb[:, ts(q4, 512)], kt[b][:, ts(q4, 512)]
                    )
                    nc.sync.dma_start(
                        v_sb[:, ts(q4, 4), :], v_r[:, ts(q4, 4), :]
                    )
                pb_sb = kv_pool.tile([D, NCHUNK], F32, tag="pb")
                nc.sync.dma_start(pb_sb[:], pb[b].rearrange("(j k) -> k j", k=128))
                return kt_sb, v_sb, pb_sb

            # pending per-group epilogues, flushed one group late so the
            # in-order DVE queue never waits on Pool
            pending = []

            def _flush_epilogue():
                pair, g, ot_ps, rb = pending.pop(0)
                rcp = rcp_pool.tile([D, 512], F32)
                with nc.allow_low_precision(reason="~51 ULP recip is plenty"):
                    nc.vector.reciprocal_approx_fast(rcp[:], rb[:])
                out_sb = osb_pool.tile([D, 512], F16)
                nc.vector.tensor_mul(out_sb[:], ot_ps[:], rcp[:])
                nc.gpsimd.dma_start(ot[pair, g], out_sb[:])

            for pair in range(PAIRS):
                b, h = pair // HPC, pair % HPC
                if h == 0:
                    kt_sb, v_sb, pb_sb = _load_kv(b)
                qt_sb = q_pool.tile([D, S], F16, tag="qt")
                for q4 in range(4):
                    nc.gpsimd.dma_start(
                        qt_sb[:, ts(q4, 512)], qt[pair][:, ts(q4, 512)]
                    )

                for g in range(NG):
                    nj = 4 * (g + 1)  # live key chunks
                    ntile = (nj + TCH - 1) // TCH
                    tiles = [
                        list(range(TCH * t, min(TCH * t + TCH, nj)))
                        for t in range(ntile)
                    ]
                    qlos = [
                        [max(0, 128 * (j - 4 * g)) for j in chunks]
                        for chunks in tiles
                    ]
                    ot_ps = ot_pool.tile([D, 512], F32)
                    acc = acc_pool.tile([D, 512], F16)

                    sts = [None] * ntile
                    pts = [None] * ntile

                    def _emit_qk(t):
                        chunks = tiles[t]
                        st = st_pool.tile([D, TCH, 512], F32)
                        sts[t] = st
                        for idx, j in enumerate(chunks):
                            u = j - 4 * g
                            qlo = qlos[t][idx]
                            nc.tensor.matmul(
                                st[:, idx, qlo:],
                                lhsT=kt_sb[:, ts(j, 128)],
                                rhs=qt_sb[:, g * 512 + qlo : (g + 1) * 512],
                                start=True,
                                stop=(u < 0),
                            )
                            if u >= 0:
                                # causal bias added on the PE itself
                                nc.tensor.matmul(
                                    st[:, idx, qlo : qlo + 128],
                                    lhsT=ident_sb[:],
                                    rhs=tri_sb[:],
                                    start=False,
                                    stop=True,
                                )

                    def _emit_exp(t):
                        chunks = tiles[t]
                        st = sts[t]
                        pt = pt_pool.tile([D, TCH, 512], F16)
                        pts[t] = pt
                        nch = len(chunks)
                        if uniform_mask:
                            qmin = min(qlos[t])
                            nc.scalar.activation(
                                pt[:, :nch, qmin:],
                                st[:, :nch, qmin:],
                                EXP,
                                scale=SCALE,
                            )
                        else:
                            for idx, j in enumerate(chunks):
                                qlo = qlos[t][idx]
                                nc.scalar.activation(
                                    pt[:, idx, qlo:],
                                    st[:, idx, qlo:],
                                    EXP,
                                    bias=pb_sb[:, j : j + 1],
                                    scale=SCALE,
                                )

                    def _emit_pv_acc(t):
                        chunks = tiles[t]
                        pt = pts[t]
                        for idx, j in enumerate(chunks):
                            qlo = qlos[t][idx]
                            nc.tensor.matmul(
                                ot_ps[:, qlo:],
                                lhsT=v_sb[:, j, :],
                                rhs=pt[:, idx, qlo:],
                                start=(j == 0),
                                stop=(j == nj - 1),
                            )
                        with nc.allow_low_precision(
                            reason="fp16 softmax denominator: consistent with "
                            "the fp16 P used in PV; ~1e-3 rel"
                        ):
                            for idx, j in enumerate(chunks):
                                qlo = qlos[t][idx]
                                if j == 0:
                                    nc.vector.tensor_copy(acc[:], pt[:, 0, :])
                                else:
                                    nc.vector.tensor_tensor(
                                        acc[:, qlo:],
                                        acc[:, qlo:],
                                        pt[:, idx, qlo:],
                                        mybir.AluOpType.add,
                                    )

                    # software-pipelined PE order: QK t0, QK t1, PV t0, ...
                    _emit_qk(0)
                    _emit_exp(0)
                    for t in range(1, ntile):
                        _emit_qk(t)
                        _emit_exp(t)
                        _emit_pv_acc(t - 1)
                    _emit_pv_acc(ntile - 1)

                    # cross-partition denominator on the idle Pool engine
                    rb = rb_pool.tile([D, 512], F32)
                    nc.gpsimd.partition_all_reduce(
                        rb[:], acc[:], channels=128, reduce_op=bass_isa.ReduceOp.add
                    )
                    pending.append((pair, g, ot_ps, rb))
                    if len(pending) > 1:
                        _flush_epilogue()
            while pending:
                _flush_epilogue()

    nc.compile()
    return nc


_NC = {}


def _get_nc(uniform_mask: bool = True):
    if uniform_mask not in _NC:
        _NC[uniform_mask] = build_module(uniform_mask)
    return _NC[uniform_mask]


def shard_inputs(q, kv, key_padding_mask):
    """Full inputs -> list of 8 per-core input maps (fp16 on device)."""
    q = np.asarray(q)
    kv = np.asarray(kv)
    mask = np.asarray(key_padding_mask)

    pbias = np.where(mask, np.float32(0.0), np.float32(NEG)).astype(np.float32)

    # in-tile causal triangle bias [k, q]: 0 if k <= q else -1e4, plus identity
    kk = np.arange(128)[:, None]
    qq = np.arange(128)[None, :]
    tri_blk = np.where(kk <= qq, np.float32(0.0), np.float32(NEG))
    tri = np.concatenate([tri_blk, np.eye(128, dtype=np.float32)], axis=1).astype(
        np.float16
    )

    in_maps = []
    for c in range(N_CORES):
        qc = q[:, :, HPC * c : HPC * (c + 1), :]  # [B, S, 4, D]
        qtc = (
            np.ascontiguousarray(np.transpose(qc, (0, 2, 3, 1)))
            .reshape(PAIRS, D, S)
            .astype(np.float16)
        )
        kc = kv[:, :, 0, c, :]  # [B, S, D]
        vc = kv[:, :, 1, c, :]  # [B, S, D]
        ktc = np.ascontiguousarray(np.transpose(kc, (0, 2, 1))).astype(np.float16)
        in_maps.append(
            {
                "qt": qtc,
                "kt": ktc,
                "v": np.ascontiguousarray(vc).astype(np.float16),
                "tri": tri,
                "pb": pbias,
            }
        )
    return in_maps


def unshard_output(results):
    """Per-core 'ot' [PAIRS, NG, D, 512] fp16 -> full [B, S, H, D] fp32."""
    out = np.empty((B, S, H, D), dtype=np.float32)
    for c in range(N_CORES):
        otc = results[c]["ot"]  # [8, 4, 128, 512]
        for pair in range(PAIRS):
            b, h = pair // HPC, HPC * c + pair % HPC
            out[b, :, h, :] = (
                np.transpose(otc[pair], (0, 2, 1)).reshape(S, D).astype(np.float32)
            )
    return out


def kernel(q, kv, key_padding_mask):
    uniform = bool(np.asarray(key_padding_mask).all())
    nc = _get_nc(uniform)
    in_maps = shard_inputs(q, kv, key_padding_mask)
    res = run_bass_kernel_spmd(nc, in_maps, core_ids=list(range(N_CORES)))
    return unshard_output(res.results)
